# revision 21
# baseline (speedup 1.0000x reference)
"""MultiHeadAttention Trainium2 kernel (8 NeuronCores).

Problem: B=2, C=512, S=2048, 8 heads (dh=64), 1x1-conv projections.

Sharding: core = (batch b, head-pair hp); b = core // 4, hp = core % 4.
Each core processes its batch's full q/k/v (channel rows hs=128*hp..+128 of
the projected tensors = heads 2hp, 2hp+1), runs attention for its 2 heads,
and computes the partial output projection Wo[:, hs:hs+128] @ attn_pair
-> (512, 2048) partial.  Host sums the 4 partials per batch (+ bo).

On-core dataflow (all matmuls bf16 in / fp32 psum out):
  kp/qp   (128=2*dh ch, 2048 s)  = WT_slice.T @ x
  vpT_h   (2048 s, 65) bf16, 65-col tiles [v(64) | ones]; ones col feeds the
          softmax denominator through the same AV matmul stream
  scores  psum (128 k, 1024 = h0 512 q | h1 512 q), 2 heads via PE row-tiles
  expS    = ACT Exp(scale=1/8) -> bf16 (no max subtraction: |scores*scale|<~6)
  AV      psum (65, 512) per head: rows 0-63 attn numerator, row 64 = denom
  norm    denom row -> DRAM bounce -> partition-broadcast -> recip -> mul
  out     partial = woT_slice.T @ [at0; at1] -> fp32 -> DRAM
"""

import numpy as np
import ml_dtypes
from contextlib import ExitStack

import concourse.bass as bass
from concourse import bacc
import concourse.mybir as mybir
import concourse.tile as tile
from concourse.bass_utils import run_bass_kernel_spmd

B = 2
C = 512
S = 2048
NH = 8
DH = C // NH            # 64
HP = 128                # channel rows per core (2 heads)
N_CORES = 8
SC = 512                # q-chunk / s-chunk size
NSC = S // SC           # 4
NKT = S // 128          # 16 k-tiles
SCALE = float(DH) ** -0.5  # 0.125

BF = mybir.dt.bfloat16
F32 = mybir.dt.float32
EXP = mybir.ActivationFunctionType.Exp
BF_NP = ml_dtypes.bfloat16

_NC_CACHE = {}


def build_nc(with_bias=True):
    nc = bacc.Bacc(None)

    xq = nc.declare_dram_parameter("xq", [C, S], BF, isOutput=False)
    xk = nc.declare_dram_parameter("xk", [C, S], BF, isOutput=False)
    xv = nc.declare_dram_parameter("xv", [C, S], BF, isOutput=False)
    wqT = nc.declare_dram_parameter("wqT", [C, HP], BF, isOutput=False)
    wkT = nc.declare_dram_parameter("wkT", [C, HP], BF, isOutput=False)
    wvT = nc.declare_dram_parameter("wvT", [C, 130], BF, isOutput=False)
    woT = nc.declare_dram_parameter("woT", [HP, C], BF, isOutput=False)
    woT_hi = nc.declare_dram_parameter("woT_hi", [64, C], BF, isOutput=False)
    bq_r = nc.declare_dram_parameter("bq_r", [1, HP], BF, isOutput=False)
    bk_r = nc.declare_dram_parameter("bk_r", [1, HP], BF, isOutput=False)
    bv_r = nc.declare_dram_parameter("bv_r", [1, 130], BF, isOutput=False)
    out_d = nc.declare_dram_parameter("out", [C, S], F32, isOutput=True)

    with tile.TileContext(nc) as tc:
        with ExitStack() as ctx:
            consts = ctx.enter_context(tc.tile_pool(name="consts", bufs=1))
            xs = ctx.enter_context(tc.tile_pool(name="xs", bufs=12))
            proj = ctx.enter_context(tc.tile_pool(name="proj", bufs=1))
            epool = ctx.enter_context(tc.tile_pool(name="epool", bufs=6))
            apool = ctx.enter_context(tc.tile_pool(name="apool", bufs=4))
            rpool = ctx.enter_context(tc.tile_pool(name="rpool", bufs=3))
            opool = ctx.enter_context(tc.tile_pool(name="opool", bufs=4))
            ps_s = ctx.enter_context(tc.tile_pool(name="ps_s", bufs=3, space="PSUM"))
            ps_av = ctx.enter_context(tc.tile_pool(name="ps_av", bufs=1, space="PSUM"))
            dscr = ctx.enter_context(tc.tile_pool(name="dscr", bufs=4, space="DRAM"))

            # Warm the ACT exp table early so the ~2.7us table load overlaps
            # the input DMA phase.
            warm = consts.tile([1, 8], F32, tag="warm")
            nc.vector.memset(warm, 0.0)
            nc.scalar.activation(out=warm, in_=warm, func=EXP)
            pewarm = consts.tile([1, 8], BF, tag="pewarm")
            nc.vector.memset(pewarm, 0.0)
            pw_ps = ps_s.tile([8, 8], F32, tag="sc", name="pw_ps")
            nc.tensor.matmul(out=pw_ps, lhsT=pewarm, rhs=pewarm, start=True, stop=True)

            # Weights / constant rows.
            wq_sb = consts.tile([128, 4, HP], BF, tag="wq")
            wk_sb = consts.tile([128, 4, HP], BF, tag="wk")
            wv_sb = consts.tile([128, 4, 130], BF, tag="wv")
            wo_sb = consts.tile([HP, C], BF, tag="wo")
            woh_sb = consts.tile([64, C], BF, tag="woh")
            ones_sb = consts.tile([1, SC], BF, tag="ones")
            bq_sb = consts.tile([1, HP], BF, tag="bq")
            bk_sb = consts.tile([1, HP], BF, tag="bk")
            bv_sb = consts.tile([1, 130], BF, tag="bv")
            nc.sync.dma_start(out=wk_sb, in_=wkT[:, :].rearrange("(t p) d -> p t d", p=128))
            nc.gpsimd.dma_start(out=wq_sb, in_=wqT[:, :].rearrange("(t p) d -> p t d", p=128))
            nc.vector.memset(ones_sb, 1.0)
            if with_bias:
                nc.sync.dma_start(out=bq_sb, in_=bq_r[:, :])
                nc.sync.dma_start(out=bk_sb, in_=bk_r[:, :])
                nc.sync.dma_start(out=bv_sb, in_=bv_r[:, :])

            # Prefetch ALL input chunks up front, alternating DMA queues.
            def stage_x(x_dram, sc, eng):
                xt = xs.tile([128, 4, SC], BF, tag="xt", name="xt")
                eng.dma_start(
                    out=xt,
                    in_=x_dram[:, :].rearrange("(t p) s -> p t s", p=128)[
                        :, :, SC * sc : SC * (sc + 1)
                    ],
                )
                return xt

            xk_t = [None] * NSC
            xq_t = [None] * NSC
            xv_t = [None] * NSC
            xk_t[0] = stage_x(xk, 0, nc.sync)
            xq_t[0] = stage_x(xq, 0, nc.gpsimd)
            xv_t[0] = stage_x(xv, 0, nc.sync)
            nc.gpsimd.dma_start(out=wv_sb, in_=wvT[:, :].rearrange("(t p) d -> p t d", p=128))
            nc.gpsimd.dma_start(out=wo_sb, in_=woT[:, :])
            nc.gpsimd.dma_start(out=woh_sb, in_=woT_hi[:, :])
            for c in range(1, NSC):
                xk_t[c] = stage_x(xk, c, nc.gpsimd)
                xv_t[c] = stage_x(xv, c, nc.sync)
            for c in range(1, NSC):
                xq_t[c] = stage_x(xq, c, nc.gpsimd)

            # Persistent projected tensors.
            kp_sb = proj.tile([128, S], BF, tag="kp")
            qp_sb = proj.tile([128, S], BF, tag="qp")
            vh_sb = [proj.tile([128, NKT * 65], BF, tag=f"vh{h}", name=f"vh{h}") for h in range(2)]
            # denominator-ones columns of vpT (col 64 of each 65-col tile)
            for h in range(2):
                v3 = vh_sb[h][:, :].rearrange("p (t c) -> p t c", c=65)
                nc.vector.memset(v3[:, :, 64:65], 1.0)

            def project_kq(xt, w_sb, b_sb, dst, sc, step=None, state={}):
                # step=None: emit the full chunk; step=(ci, ps): emit one
                # c-tile accumulation step, evac after the last.
                if step is None:
                    ps = ps_s.tile([128, SC], F32, tag="sc", name="ps")
                    cis = range(4)
                else:
                    ci, ps = step
                    cis = [ci]
                for ci in cis:
                    nc.tensor.matmul(
                        out=ps, lhsT=w_sb[:, ci, :], rhs=xt[:, ci, :],
                        start=(ci == 0), stop=(ci == 3 and not with_bias),
                    )
                if step is not None and step[0] < 3:
                    return
                if with_bias:
                    nc.tensor.matmul(
                        out=ps, lhsT=b_sb, rhs=ones_sb, start=False, stop=True,
                    )
                nc.vector.tensor_copy(out=dst[:, SC * sc : SC * (sc + 1)], in_=ps)

            def project_v_tile(T):
                # vpT s-tile T: psum (128 s, 130) = x_v_tile.T @ WvT2
                xt = xv_t[T // 4]
                j = T % 4
                ps = ps_s.tile([128, SC], F32, tag="sc", name="ps")
                psv = ps[:, 0:130]
                for ci in range(4):
                    nc.tensor.matmul(
                        out=psv, lhsT=xt[:, ci, 128 * j : 128 * (j + 1)],
                        rhs=wv_sb[:, ci, :],
                        start=(ci == 0), stop=(ci == 3 and not with_bias),
                    )
                if with_bias:
                    nc.tensor.matmul(
                        out=psv, lhsT=ones_sb[:, 0:128], rhs=bv_sb,
                        start=False, stop=True,
                    )
                nc.vector.tensor_copy(
                    out=vh_sb[0][:, 65 * T : 65 * T + 64], in_=psv[:, 0:64]
                )
                nc.vector.tensor_copy(
                    out=vh_sb[1][:, 65 * T : 65 * T + 64], in_=psv[:, 65:129]
                )

            def emit_kloop(qc, av, interleave=None):
                qsl = slice(SC * qc, SC * (qc + 1))
                for t in range(NKT):
                    if interleave is not None:
                        interleave(t)
                    ksl = slice(128 * t, 128 * (t + 1))
                    st = ps_s.tile([128, 2 * SC], F32, tag="sc", name="st")
                    nc.tensor.matmul(
                        out=st[:, 0:SC], lhsT=kp_sb[0:64, ksl], rhs=qp_sb[0:64, qsl],
                        start=True, stop=True, tile_position=(0, 0),
                    )
                    nc.tensor.matmul(
                        out=st[:, SC : 2 * SC], lhsT=kp_sb[64:128, ksl],
                        rhs=qp_sb[64:128, qsl],
                        start=True, stop=True, tile_position=(64, 0),
                    )
                    ex = epool.tile([128, 2 * SC], BF, tag="ex", name="ex")
                    nc.scalar.activation(out=ex, in_=st, func=EXP, scale=SCALE)
                    vsl = slice(65 * t, 65 * (t + 1))
                    for h in range(2):
                        exh = ex[:, SC * h : SC * (h + 1)]
                        nc.tensor.matmul(
                            out=av[h], lhsT=vh_sb[h][:, vsl], rhs=exh,
                            start=(t == 0), stop=(t == NKT - 1),
                        )

            def emit_norm(qc, av, stack=True):
                # Interleave the two head chains; each: one fp32 copy (frees
                # the AV psum slot), DRAM bounce, partition-broadcast, recip,
                # normalize-multiply.
                avc, dsc, sbb, rbb = [], [], [], []
                for h in range(2):
                    avc.append(rpool.tile([65, SC], F32, tag=f"avc{h}", name="avc"))
                    nc.vector.tensor_copy(out=avc[h], in_=av[h])
                for h in range(2):
                    dsc.append(dscr.tile([1, SC], F32, tag=f"dsc{h}", name="dsc"))
                    nc.sync.dma_start(out=dsc[h], in_=avc[h][64:65, :])
                for h in range(2):
                    sbb.append(rpool.tile([64, SC], F32, tag=f"sbb{h}", name="sbb"))
                    nc.sync.dma_start(
                        out=sbb[h],
                        in_=bass.AP(
                            tensor=dsc[h].tensor, offset=dsc[h].offset,
                            ap=[[0, 64], *dsc[h].ap[1:]],
                        ),
                    )
                for h in range(2):
                    rbb.append(rpool.tile([64, SC], F32, tag=f"rbb{h}", name="rbb"))
                    nc.vector.reciprocal_approx_fast(out=rbb[h], in_=sbb[h])
                if stack:
                    atile = apool.tile([128, SC], BF, tag="at", name="atile")
                    nc.vector.tensor_mul(
                        out=atile[0:64, :], in0=avc[0][0:64, :], in1=rbb[0]
                    )
                    at1 = apool.tile([64, SC], BF, tag="at1", name="at1")
                    nc.vector.tensor_mul(out=at1, in0=avc[1][0:64, :], in1=rbb[1])
                    nc.sync.dma_start(out=atile[64:128, :], in_=at1)
                    return atile
                at0 = apool.tile([64, SC], BF, tag="at", name="at0")
                at1 = apool.tile([64, SC], BF, tag="at1", name="at1")
                nc.vector.tensor_mul(out=at0, in0=avc[0][0:64, :], in1=rbb[0])
                nc.vector.tensor_mul(out=at1, in0=avc[1][0:64, :], in1=rbb[1])
                return (at0, at1)

            def emit_outproj(qc, atile, last=False, act_evac=False):
                qsl = slice(SC * qc, SC * (qc + 1))
                for ot in range(4):
                    osl = slice(128 * ot, 128 * (ot + 1))
                    ps = ps_s.tile([128, SC], F32, tag="sc", name="ps_o")
                    if last:
                        nc.tensor.matmul(
                            out=ps, lhsT=wo_sb[0:64, osl], rhs=atile[0],
                            start=True, stop=False,
                        )
                        nc.tensor.matmul(
                            out=ps, lhsT=woh_sb[:, osl], rhs=atile[1],
                            start=False, stop=True,
                        )
                    else:
                        nc.tensor.matmul(
                            out=ps, lhsT=wo_sb[:, osl], rhs=atile,
                            start=True, stop=True,
                        )
                    osb = opool.tile([128, SC], F32, tag="ob", name="osb")
                    if act_evac and ot % 2 == 1:
                        nc.scalar.activation(
                            out=osb, in_=ps, func=mybir.ActivationFunctionType.Copy
                        )
                    else:
                        nc.vector.tensor_copy(out=osb, in_=ps)
                    nc.sync.dma_start(
                        out=out_d[128 * ot : 128 * (ot + 1), qsl], in_=osb
                    )

            # Prologue: only what k-tile 0 of q-chunk 0 needs; kp chunks and
            # vpT tiles stream between score tiles of q-chunk 0.
            project_kq(xk_t[0], wk_sb, bk_sb, kp_sb, 0)
            project_kq(xq_t[0], wq_sb, bq_sb, qp_sb, 0)

            kp_ps = {}

            def interleave_proj(t):
                c = t // 4 + 1
                if c < NSC and t % 4 == 2:
                    kp_ps[c] = ps_s.tile([128, SC], F32, tag="sc", name="kpps")
                    project_kq(xk_t[c], wk_sb, bk_sb, kp_sb, c, step=(0, kp_ps[c]))
                    project_kq(xk_t[c], wk_sb, bk_sb, kp_sb, c, step=(1, kp_ps[c]))
                elif c < NSC and t % 4 == 3:
                    project_kq(xk_t[c], wk_sb, bk_sb, kp_sb, c, step=(2, kp_ps[c]))
                    project_kq(xk_t[c], wk_sb, bk_sb, kp_sb, c, step=(3, kp_ps[c]))
                project_v_tile(t)

            avs = {}
            ats = {}
            avs[0] = [
                ps_av.tile([65, SC], F32, tag=f"av{h}", name=f"av{h}")
                for h in range(2)
            ]
            emit_kloop(0, avs[0], interleave=interleave_proj)
            for c in range(1, NSC):
                project_kq(xq_t[c], wq_sb, bq_sb, qp_sb, c)
            ats[0] = emit_norm(0, avs[0])
            for qc in range(1, NSC):
                avs[qc] = [
                    ps_av.tile([65, SC], F32, tag=f"av{h}", name=f"av{h}")
                    for h in range(2)
                ]
                emit_kloop(qc, avs[qc])
                ats[qc] = emit_norm(qc, avs[qc], stack=(qc != NSC - 1))
                emit_outproj(qc - 1, ats[qc - 1], act_evac=(qc == NSC - 1))
            emit_outproj(NSC - 1, ats[NSC - 1], last=True, act_evac=True)

    nc.compile()
    return nc


def make_in_maps(q, k, v, Wq, bq, Wk, bk, Wv, bv, Wo, bo):
    q, k, v = (np.asarray(t, np.float32) for t in (q, k, v))
    Wq, Wk, Wv, Wo = (np.asarray(t, np.float32) for t in (Wq, Wk, Wv, Wo))
    bq, bk, bv = (np.asarray(t, np.float32) for t in (bq, bk, bv))
    in_maps = []
    for core in range(N_CORES):
        b, hp = core // 4, core % 4
        hs = slice(HP * hp, HP * (hp + 1))
        wvs = Wv[hs, :].T  # (C, 128)
        wv2 = np.zeros((C, 130), np.float32)
        wv2[:, 0:64] = wvs[:, 0:64]
        wv2[:, 65:129] = wvs[:, 64:128]
        bv2 = np.zeros(130, np.float32)
        bv2[0:64] = bv[hs][0:64]
        bv2[64] = 1.0
        bv2[65:129] = bv[hs][64:128]
        bv2[129] = 1.0
        in_maps.append({
            "xq": np.ascontiguousarray(q[b, :, 0, :]).astype(BF_NP),
            "xk": np.ascontiguousarray(k[b, :, 0, :]).astype(BF_NP),
            "xv": np.ascontiguousarray(v[b, :, 0, :]).astype(BF_NP),
            "wqT": np.ascontiguousarray(Wq[hs, :].T).astype(BF_NP),
            "wkT": np.ascontiguousarray(Wk[hs, :].T).astype(BF_NP),
            "wvT": wv2.astype(BF_NP),
            "woT": np.ascontiguousarray(Wo[:, hs].T).astype(BF_NP),
            "woT_hi": np.ascontiguousarray(Wo[:, 128 * hp + 64 : 128 * hp + 128].T).astype(BF_NP),
            "bq_r": bq[hs].reshape(1, HP).astype(BF_NP),
            "bk_r": bk[hs].reshape(1, HP).astype(BF_NP),
            "bv_r": bv2.reshape(1, 130).astype(BF_NP),
        })
    return in_maps


def assemble_output(results, bo):
    bo = np.asarray(bo, np.float32)
    out = np.zeros((B, C, 1, S), np.float32)
    for b in range(B):
        acc = np.zeros((C, S), np.float32)
        for hp in range(4):
            acc += np.asarray(results[b * 4 + hp]["out"], np.float32)
        out[b, :, 0, :] = acc + bo[:, None]
    return out


def kernel(q, k, v, Wq, bq, Wk, bk, Wv, bv, Wo, bo):
    zero_bias = not (
        np.any(np.asarray(bq)) or np.any(np.asarray(bk)) or np.any(np.asarray(bv))
    )
    key = not zero_bias
    if key not in _NC_CACHE:
        _NC_CACHE[key] = build_nc(with_bias=key)
    nc = _NC_CACHE[key]
    in_maps = make_in_maps(q, k, v, Wq, bq, Wk, bk, Wv, bv, Wo, bo)
    res = run_bass_kernel_spmd(nc, in_maps, list(range(N_CORES)))
    return assemble_output(res.results, bo)


# revision 22
# speedup vs baseline: 1.0451x; 1.0451x over previous
"""MultiHeadAttention Trainium2 kernel (8 NeuronCores).

Problem: B=2, C=512, S=2048, 8 heads (dh=64), 1x1-conv projections.

Sharding: core = (batch b, head-pair hp); b = core // 4, hp = core % 4.
Each core processes its batch's full q/k/v (channel rows hs=128*hp..+128 of
the projected tensors = heads 2hp, 2hp+1), runs attention for its 2 heads,
and computes the partial output projection Wo[:, hs:hs+128] @ attn_pair
-> (512, 2048) partial.  Host sums the 4 partials per batch (+ bo).

On-core dataflow (all matmuls bf16 in / fp32 psum out):
  kp/qp   (128=2*dh ch, 2048 s)  = WT_slice.T @ x
  vpT_h   (2048 s, 65) bf16, 65-col tiles [v(64) | ones]; ones col feeds the
          softmax denominator through the same AV matmul stream
  scores  psum (128 k, 1024 = h0 512 q | h1 512 q), 2 heads via PE row-tiles
  expS    = ACT Exp(scale=1/8) -> bf16 (no max subtraction: |scores*scale|<~6)
  AV      psum (65, 512) per head: rows 0-63 attn numerator, row 64 = denom
  norm    denom row -> DRAM bounce -> partition-broadcast -> recip -> mul
  out     partial = woT_slice.T @ [at0; at1] -> fp32 -> DRAM
"""

import numpy as np
import ml_dtypes
from contextlib import ExitStack

import concourse.bass as bass
from concourse import bacc
import concourse.mybir as mybir
import concourse.tile as tile
from concourse.bass_utils import run_bass_kernel_spmd

B = 2
C = 512
S = 2048
NH = 8
DH = C // NH            # 64
HP = 128                # channel rows per core (2 heads)
N_CORES = 8
SC = 512                # q-chunk / s-chunk size
NSC = S // SC           # 4
NKT = S // 128          # 16 k-tiles
SCALE = float(DH) ** -0.5  # 0.125

BF = mybir.dt.bfloat16
F32 = mybir.dt.float32
EXP = mybir.ActivationFunctionType.Exp
BF_NP = ml_dtypes.bfloat16

_NC_CACHE = {}


def build_nc(with_bias=True):
    nc = bacc.Bacc(None)

    xq = nc.declare_dram_parameter("xq", [C, S], BF, isOutput=False)
    xk = nc.declare_dram_parameter("xk", [C, S], BF, isOutput=False)
    xv = nc.declare_dram_parameter("xv", [C, S], BF, isOutput=False)
    wqT = nc.declare_dram_parameter("wqT", [C, HP], BF, isOutput=False)
    wkT = nc.declare_dram_parameter("wkT", [C, HP], BF, isOutput=False)
    wvT = nc.declare_dram_parameter("wvT", [C, 130], BF, isOutput=False)
    woT = nc.declare_dram_parameter("woT", [HP, C], BF, isOutput=False)
    woT_hi = nc.declare_dram_parameter("woT_hi", [64, C], BF, isOutput=False)
    bq_r = nc.declare_dram_parameter("bq_r", [1, HP], BF, isOutput=False)
    bk_r = nc.declare_dram_parameter("bk_r", [1, HP], BF, isOutput=False)
    bv_r = nc.declare_dram_parameter("bv_r", [1, 130], BF, isOutput=False)
    out_d = nc.declare_dram_parameter("out", [C, S], F32, isOutput=True)

    with tile.TileContext(nc) as tc:
        with ExitStack() as ctx:
            consts = ctx.enter_context(tc.tile_pool(name="consts", bufs=1))
            xs = ctx.enter_context(tc.tile_pool(name="xs", bufs=12))
            proj = ctx.enter_context(tc.tile_pool(name="proj", bufs=1))
            epool = ctx.enter_context(tc.tile_pool(name="epool", bufs=6))
            apool = ctx.enter_context(tc.tile_pool(name="apool", bufs=4))
            rpool = ctx.enter_context(tc.tile_pool(name="rpool", bufs=3))
            opool = ctx.enter_context(tc.tile_pool(name="opool", bufs=4))
            ps_s = ctx.enter_context(tc.tile_pool(name="ps_s", bufs=3, space="PSUM"))
            ps_av = ctx.enter_context(tc.tile_pool(name="ps_av", bufs=1, space="PSUM"))
            dscr = ctx.enter_context(tc.tile_pool(name="dscr", bufs=4, space="DRAM"))

            # Warm the ACT exp table early so the ~2.7us table load overlaps
            # the input DMA phase.
            warm = consts.tile([1, 8], F32, tag="warm")
            nc.vector.memset(warm, 0.0)
            nc.scalar.activation(out=warm, in_=warm, func=EXP)
            pewarm = consts.tile([1, 8], BF, tag="pewarm")
            nc.vector.memset(pewarm, 0.0)
            pw_ps = ps_s.tile([8, 8], F32, tag="sc", name="pw_ps")
            nc.tensor.matmul(out=pw_ps, lhsT=pewarm, rhs=pewarm, start=True, stop=True)

            # Weights / constant rows.
            wq_sb = consts.tile([128, 4, HP], BF, tag="wq")
            wk_sb = consts.tile([128, 4, HP], BF, tag="wk")
            wv_sb = consts.tile([128, 4, 130], BF, tag="wv")
            wo_sb = consts.tile([HP, C], BF, tag="wo")
            woh_sb = consts.tile([64, C], BF, tag="woh")
            ones_sb = consts.tile([1, SC], BF, tag="ones")
            bq_sb = consts.tile([1, HP], BF, tag="bq")
            bk_sb = consts.tile([1, HP], BF, tag="bk")
            bv_sb = consts.tile([1, 130], BF, tag="bv")
            nc.sync.dma_start(out=wk_sb, in_=wkT[:, :].rearrange("(t p) d -> p t d", p=128))
            nc.gpsimd.dma_start(out=wq_sb, in_=wqT[:, :].rearrange("(t p) d -> p t d", p=128))
            nc.vector.memset(ones_sb, 1.0)
            if with_bias:
                nc.sync.dma_start(out=bq_sb, in_=bq_r[:, :])
                nc.sync.dma_start(out=bk_sb, in_=bk_r[:, :])
                nc.sync.dma_start(out=bv_sb, in_=bv_r[:, :])

            # Prefetch ALL input chunks up front, alternating DMA queues.
            def stage_x(x_dram, sc, eng):
                xt = xs.tile([128, 4, SC], BF, tag="xt", name="xt")
                eng.dma_start(
                    out=xt,
                    in_=x_dram[:, :].rearrange("(t p) s -> p t s", p=128)[
                        :, :, SC * sc : SC * (sc + 1)
                    ],
                )
                return xt

            xk_t = [None] * NSC
            xq_t = [None] * NSC
            xv_t = [None] * NSC
            xk_t[0] = stage_x(xk, 0, nc.sync)
            xq_t[0] = stage_x(xq, 0, nc.gpsimd)
            xv_t[0] = stage_x(xv, 0, nc.sync)
            nc.gpsimd.dma_start(out=wv_sb, in_=wvT[:, :].rearrange("(t p) d -> p t d", p=128))
            nc.gpsimd.dma_start(out=wo_sb, in_=woT[:, :])
            nc.gpsimd.dma_start(out=woh_sb, in_=woT_hi[:, :])
            for c in range(1, NSC):
                xk_t[c] = stage_x(xk, c, nc.gpsimd)
                xv_t[c] = stage_x(xv, c, nc.sync)
            for c in range(1, NSC):
                xq_t[c] = stage_x(xq, c, nc.gpsimd)

            # Persistent projected tensors.
            kp_sb = proj.tile([128, S], BF, tag="kp")
            qp_sb = proj.tile([128, S], BF, tag="qp")
            vh_sb = [proj.tile([128, NKT * 65], BF, tag=f"vh{h}", name=f"vh{h}") for h in range(2)]
            # denominator-ones columns of vpT (col 64 of each 65-col tile)
            for h in range(2):
                v3 = vh_sb[h][:, :].rearrange("p (t c) -> p t c", c=65)
                nc.vector.memset(v3[:, :, 64:65], 1.0)

            def project_kq(xt, w_sb, b_sb, dst, sc, step=None, state={}):
                # step=None: emit the full chunk; step=(ci, ps): emit one
                # c-tile accumulation step, evac after the last.
                if step is None:
                    ps = ps_s.tile([128, SC], F32, tag="sc", name="ps")
                    cis = range(4)
                else:
                    ci, ps = step
                    cis = [ci]
                for ci in cis:
                    nc.tensor.matmul(
                        out=ps, lhsT=w_sb[:, ci, :], rhs=xt[:, ci, :],
                        start=(ci == 0), stop=(ci == 3 and not with_bias),
                    )
                if step is not None and step[0] < 3:
                    return
                if with_bias:
                    nc.tensor.matmul(
                        out=ps, lhsT=b_sb, rhs=ones_sb, start=False, stop=True,
                    )
                nc.vector.tensor_copy(out=dst[:, SC * sc : SC * (sc + 1)], in_=ps)

            def project_v_tile(T):
                # vpT s-tile T: psum (128 s, 130) = x_v_tile.T @ WvT2
                xt = xv_t[T // 4]
                j = T % 4
                ps = ps_s.tile([128, SC], F32, tag="sc", name="ps")
                psv = ps[:, 0:130]
                for ci in range(4):
                    nc.tensor.matmul(
                        out=psv, lhsT=xt[:, ci, 128 * j : 128 * (j + 1)],
                        rhs=wv_sb[:, ci, :],
                        start=(ci == 0), stop=(ci == 3 and not with_bias),
                    )
                if with_bias:
                    nc.tensor.matmul(
                        out=psv, lhsT=ones_sb[:, 0:128], rhs=bv_sb,
                        start=False, stop=True,
                    )
                nc.vector.tensor_copy(
                    out=vh_sb[0][:, 65 * T : 65 * T + 64], in_=psv[:, 0:64]
                )
                nc.vector.tensor_copy(
                    out=vh_sb[1][:, 65 * T : 65 * T + 64], in_=psv[:, 65:129]
                )

            def emit_kloop(qc, av, interleave=None):
                qsl = slice(SC * qc, SC * (qc + 1))
                for t in range(NKT):
                    if interleave is not None:
                        interleave(t)
                    ksl = slice(128 * t, 128 * (t + 1))
                    st = ps_s.tile([128, 2 * SC], F32, tag="sc", name="st")
                    nc.tensor.matmul(
                        out=st[:, 0:SC], lhsT=kp_sb[0:64, ksl], rhs=qp_sb[0:64, qsl],
                        start=True, stop=True, tile_position=(0, 0),
                    )
                    nc.tensor.matmul(
                        out=st[:, SC : 2 * SC], lhsT=kp_sb[64:128, ksl],
                        rhs=qp_sb[64:128, qsl],
                        start=True, stop=True, tile_position=(64, 0),
                    )
                    ex = epool.tile([128, 2 * SC], BF, tag="ex", name="ex")
                    nc.scalar.activation(out=ex, in_=st, func=EXP, scale=SCALE)
                    vsl = slice(65 * t, 65 * (t + 1))
                    for h in range(2):
                        exh = ex[:, SC * h : SC * (h + 1)]
                        nc.tensor.matmul(
                            out=av[h], lhsT=vh_sb[h][:, vsl], rhs=exh,
                            start=(t == 0), stop=(t == NKT - 1),
                        )

            def emit_norm(qc, av, stack=True):
                # Interleave the two head chains; each: one fp32 copy (frees
                # the AV psum slot), DRAM bounce, partition-broadcast, recip,
                # normalize-multiply.
                avc, dsc, sbb, rbb = [], [], [], []
                for h in range(2):
                    avc.append(rpool.tile([65, SC], F32, tag=f"avc{h}", name="avc"))
                    nc.vector.tensor_copy(out=avc[h], in_=av[h])
                for h in range(2):
                    dsc.append(dscr.tile([1, SC], F32, tag=f"dsc{h}", name="dsc"))
                    nc.sync.dma_start(out=dsc[h], in_=avc[h][64:65, :])
                for h in range(2):
                    sbb.append(rpool.tile([64, SC], F32, tag=f"sbb{h}", name="sbb"))
                    nc.sync.dma_start(
                        out=sbb[h],
                        in_=bass.AP(
                            tensor=dsc[h].tensor, offset=dsc[h].offset,
                            ap=[[0, 64], *dsc[h].ap[1:]],
                        ),
                    )
                for h in range(2):
                    rbb.append(rpool.tile([64, SC], F32, tag=f"rbb{h}", name="rbb"))
                    nc.vector.reciprocal_approx_fast(out=rbb[h], in_=sbb[h])
                if stack:
                    atile = apool.tile([128, SC], BF, tag="at", name="atile")
                    nc.vector.tensor_mul(
                        out=atile[0:64, :], in0=avc[0][0:64, :], in1=rbb[0]
                    )
                    at1 = apool.tile([64, SC], BF, tag="at1", name="at1")
                    nc.vector.tensor_mul(out=at1, in0=avc[1][0:64, :], in1=rbb[1])
                    nc.sync.dma_start(out=atile[64:128, :], in_=at1)
                    return atile
                at0 = apool.tile([64, SC], BF, tag="at", name="at0")
                at1 = apool.tile([64, SC], BF, tag="at1", name="at1")
                nc.vector.tensor_mul(out=at0, in0=avc[0][0:64, :], in1=rbb[0])
                nc.vector.tensor_mul(out=at1, in0=avc[1][0:64, :], in1=rbb[1])
                return (at0, at1)

            def emit_outproj(qc, atile, last=False, act_evac=False):
                qsl = slice(SC * qc, SC * (qc + 1))
                for ot in range(4):
                    osl = slice(128 * ot, 128 * (ot + 1))
                    ps = ps_s.tile([128, SC], F32, tag="sc", name="ps_o")
                    if last:
                        nc.tensor.matmul(
                            out=ps, lhsT=wo_sb[0:64, osl], rhs=atile[0],
                            start=True, stop=False,
                        )
                        nc.tensor.matmul(
                            out=ps, lhsT=woh_sb[:, osl], rhs=atile[1],
                            start=False, stop=True,
                        )
                    else:
                        nc.tensor.matmul(
                            out=ps, lhsT=wo_sb[:, osl], rhs=atile,
                            start=True, stop=True,
                        )
                    osb = opool.tile([128, SC], F32, tag="ob", name="osb")
                    if act_evac and ot % 2 == 1:
                        nc.scalar.activation(
                            out=osb, in_=ps, func=mybir.ActivationFunctionType.Copy
                        )
                    else:
                        nc.vector.tensor_copy(out=osb, in_=ps)
                    nc.sync.dma_start(
                        out=out_d[128 * ot : 128 * (ot + 1), qsl], in_=osb
                    )

            # Prologue: only what k-tile 0 of q-chunk 0 needs; kp chunks and
            # vpT tiles stream between score tiles of q-chunk 0.
            project_kq(xk_t[0], wk_sb, bk_sb, kp_sb, 0)
            project_kq(xq_t[0], wq_sb, bq_sb, qp_sb, 0)

            def interleave_proj(t):
                if t % 4 == 0 and t > 0:
                    project_kq(xk_t[t // 4], wk_sb, bk_sb, kp_sb, t // 4)
                project_v_tile(t)

            avs = {}
            ats = {}
            avs[0] = [
                ps_av.tile([65, SC], F32, tag=f"av{h}", name=f"av{h}")
                for h in range(2)
            ]
            emit_kloop(0, avs[0], interleave=interleave_proj)
            for c in range(1, NSC):
                project_kq(xq_t[c], wq_sb, bq_sb, qp_sb, c)
            ats[0] = emit_norm(0, avs[0])
            for qc in range(1, NSC):
                avs[qc] = [
                    ps_av.tile([65, SC], F32, tag=f"av{h}", name=f"av{h}")
                    for h in range(2)
                ]
                emit_kloop(qc, avs[qc])
                ats[qc] = emit_norm(qc, avs[qc], stack=(qc != NSC - 1))
                emit_outproj(qc - 1, ats[qc - 1], act_evac=(qc == NSC - 1))
            emit_outproj(NSC - 1, ats[NSC - 1], last=True, act_evac=True)

    nc.compile()
    return nc


def make_in_maps(q, k, v, Wq, bq, Wk, bk, Wv, bv, Wo, bo):
    q, k, v = (np.asarray(t, np.float32) for t in (q, k, v))
    Wq, Wk, Wv, Wo = (np.asarray(t, np.float32) for t in (Wq, Wk, Wv, Wo))
    bq, bk, bv = (np.asarray(t, np.float32) for t in (bq, bk, bv))
    in_maps = []
    for core in range(N_CORES):
        b, hp = core // 4, core % 4
        hs = slice(HP * hp, HP * (hp + 1))
        wvs = Wv[hs, :].T  # (C, 128)
        wv2 = np.zeros((C, 130), np.float32)
        wv2[:, 0:64] = wvs[:, 0:64]
        wv2[:, 65:129] = wvs[:, 64:128]
        bv2 = np.zeros(130, np.float32)
        bv2[0:64] = bv[hs][0:64]
        bv2[64] = 1.0
        bv2[65:129] = bv[hs][64:128]
        bv2[129] = 1.0
        in_maps.append({
            "xq": np.ascontiguousarray(q[b, :, 0, :]).astype(BF_NP),
            "xk": np.ascontiguousarray(k[b, :, 0, :]).astype(BF_NP),
            "xv": np.ascontiguousarray(v[b, :, 0, :]).astype(BF_NP),
            "wqT": np.ascontiguousarray(Wq[hs, :].T).astype(BF_NP),
            "wkT": np.ascontiguousarray(Wk[hs, :].T).astype(BF_NP),
            "wvT": wv2.astype(BF_NP),
            "woT": np.ascontiguousarray(Wo[:, hs].T).astype(BF_NP),
            "woT_hi": np.ascontiguousarray(Wo[:, 128 * hp + 64 : 128 * hp + 128].T).astype(BF_NP),
            "bq_r": bq[hs].reshape(1, HP).astype(BF_NP),
            "bk_r": bk[hs].reshape(1, HP).astype(BF_NP),
            "bv_r": bv2.reshape(1, 130).astype(BF_NP),
        })
    return in_maps


def assemble_output(results, bo):
    bo = np.asarray(bo, np.float32)
    out = np.zeros((B, C, 1, S), np.float32)
    for b in range(B):
        acc = np.zeros((C, S), np.float32)
        for hp in range(4):
            acc += np.asarray(results[b * 4 + hp]["out"], np.float32)
        out[b, :, 0, :] = acc + bo[:, None]
    return out


def kernel(q, k, v, Wq, bq, Wk, bk, Wv, bv, Wo, bo):
    zero_bias = not (
        np.any(np.asarray(bq)) or np.any(np.asarray(bk)) or np.any(np.asarray(bv))
    )
    key = not zero_bias
    if key not in _NC_CACHE:
        _NC_CACHE[key] = build_nc(with_bias=key)
    nc = _NC_CACHE[key]
    in_maps = make_in_maps(q, k, v, Wq, bq, Wk, bk, Wv, bv, Wo, bo)
    res = run_bass_kernel_spmd(nc, in_maps, list(range(N_CORES)))
    return assemble_output(res.results, bo)
